# revision 1
# baseline (speedup 1.0000x reference)
"""Trainium2 8-core tensor-parallel transformer block (RMSNorm + RoPE causal
attention + SwiGLU FFN).

Sharding (SPMD, identical program on 8 cores, per-core data via in_maps):
  - attention: heads sharded (2 heads/core); q/k/v projections row-sharded
  - o-projection and FFN down-projection: token-sharded after an AllToAll
  - FFN up (w1/w3): dff-sharded
  - norms: token-sharded, AllGather of normalized activations (bf16)

Collectives: AG(h1) -> attn -> A2A(head outputs) -> o-proj(own tokens)
             -> AG(h2) -> ffn up -> A2A(activations) -> down-proj(own tokens)

All matmuls bf16 x bf16 -> fp32 PSUM. Residual spine fp32.
RoPE is applied as q_rot = X*C + X_swap*S where X_swap comes from a second
matmul with row-swapped weights (avoids partition-crossing vector ops);
q/k rows are host-permuted to [h0-even, h0-odd, h1-even, h1-odd], which
leaves q.k dot products unchanged.
Softmax skips the running max (|scores| < 4 for this problem's scale) and
gets its denominator for free from a ones-column appended to V.
"""

from contextlib import ExitStack

import numpy as np
import ml_dtypes

import concourse.mybir as mybir
import concourse.tile as tile
from concourse import bacc
from concourse.bass import ds, ts
from concourse.bass_utils import run_bass_kernel_spmd

B, S, D, H, DK, DFF = 2, 2048, 1024, 16, 64, 4096
THETA, EPS = 10000.0, 1e-5
N = 8            # cores
T = B * S // N   # tokens per core (512)
HPC = H // N     # heads per core (2)
DFFC = DFF // N  # dff per core (512)

F32 = mybir.dt.float32
BF16 = mybir.dt.bfloat16
BF16NP = ml_dtypes.bfloat16
AF = mybir.ActivationFunctionType

_NC = None


def _build():
    nc = bacc.Bacc("TRN2", target_bir_lowering=False)

    # ---- I/O ----
    x_in = nc.dram_tensor("x", [T, D], F32, kind="ExternalInput")
    wqT = nc.dram_tensor("wqT", [D, 128], BF16, kind="ExternalInput")
    wkT = nc.dram_tensor("wkT", [D, 128], BF16, kind="ExternalInput")
    perm_in = nc.dram_tensor("rope_perm", [128, 128], BF16, kind="ExternalInput")
    ident_in = nc.dram_tensor("ident", [128, 128], BF16, kind="ExternalInput")
    wvT = nc.dram_tensor("wvT", [D, 128], BF16, kind="ExternalInput")
    woT = nc.dram_tensor("woT", [D, D], BF16, kind="ExternalInput")
    w1T = nc.dram_tensor("w1T", [D, DFFC], BF16, kind="ExternalInput")
    w3T = nc.dram_tensor("w3T", [D, DFFC], BF16, kind="ExternalInput")
    w2T = nc.dram_tensor("w2T", [DFF, D], BF16, kind="ExternalInput")
    trigC = nc.dram_tensor("trigC", [128, S], F32, kind="ExternalInput")
    trigS = nc.dram_tensor("trigS", [128, S], F32, kind="ExternalInput")
    mask_in = nc.dram_tensor("mask", [128, 128], BF16, kind="ExternalInput")
    out_ext = nc.dram_tensor("out", [T, D], F32, kind="ExternalOutput")

    # ---- internal DRAM ----
    h1_tm = nc.dram_tensor("h1_tm", [T, D], BF16)
    h1t_in = nc.dram_tensor("h1t_in", [D, T], BF16)
    h1t_ag = nc.dram_tensor("h1t_ag", [N * D, T], BF16, addr_space="Shared")
    h2_tm = nc.dram_tensor("h2_tm", [T, D], BF16)
    h2t_in = nc.dram_tensor("h2t_in", [D, T], BF16)
    h2t_ag = nc.dram_tensor("h2t_ag", [N * D, T], BF16, addr_space="Shared")
    o_a2a_in = nc.dram_tensor("o_a2a_in", [N * 128, T], BF16)
    o_a2a_out = nc.dram_tensor("o_a2a_out", [N * 128, T], BF16)
    s_a2a_in_a = nc.dram_tensor("s_a2a_in_a", [N * DFFC // 2, T], BF16)
    s_a2a_in_b = nc.dram_tensor("s_a2a_in_b", [N * DFFC // 2, T], BF16)
    s_a2a_out_a = nc.dram_tensor("s_a2a_out_a", [N * DFFC // 2, T], BF16)
    s_a2a_out_b = nc.dram_tensor("s_a2a_out_b", [N * DFFC // 2, T], BF16)

    rg = [list(range(N))]

    with tile.TileContext(nc) as tc, ExitStack() as stack:
        consts = stack.enter_context(tc.tile_pool(name="consts", bufs=1))
        persist = stack.enter_context(tc.tile_pool(name="persist", bufs=1))
        wpool = stack.enter_context(tc.tile_pool(name="wpool", bufs=1))
        big = stack.enter_context(tc.tile_pool(name="big", bufs=1))

        # x first — it heads the critical path into norm1 + AG
        xm_sb = persist.tile([128, 4, D], F32)   # x, later mid (x + attn)
        nc.sync.dma_start(out=xm_sb, in_=x_in[:].rearrange("(t p) d -> p t d", p=128))

        ones_sb = consts.tile([1, 64], BF16)
        nc.vector.memset(ones_sb, 1.0)
        ident_sb = consts.tile([128, 128], BF16)
        nc.sync.dma_start(out=ident_sb, in_=ident_in[:])
        eps_sb = consts.tile([128, 1], F32)
        nc.vector.memset(eps_sb, EPS)

        def load_w(name, dram, cols):
            t = wpool.tile([128, 8, cols], BF16, tag=name)
            nc.scalar.dma_start(out=t, in_=dram[:].rearrange("(c p) f -> p c f", p=128))
            return t

        qT_sb = persist.tile([128, B * S], BF16)
        kT_sb = persist.tile([128, B * S], BF16)
        v_sb = persist.tile([128, 32, 130], BF16)
        oT_sb = persist.tile([128, B * S], BF16)

        # ---- norm helper: src [128,4,D] f32 -> tm/tin DRAM + AG ----
        def rmsnorm_to_ag(src_sb, tm_dram, tin_dram, ag_out_dram):
            with (
                tc.tile_pool(name="norm", bufs=2) as npool,
                tc.tile_pool(name="norm_ps", bufs=2, space="PSUM") as nps,
            ):
                hT_own = npool.tile([128, 8, T], BF16, tag="hT_own")
                for tt4 in range(4):
                    xsl = src_sb[:, tt4, :]
                    stats = npool.tile([128, 2, 6], F32, tag="stats")
                    nc.vector.bn_stats(out=stats[:, 0, :], in_=xsl[:, 0:512])
                    nc.vector.bn_stats(out=stats[:, 1, :], in_=xsl[:, 512:1024])
                    mv = npool.tile([128, 2], F32, tag="mv")
                    nc.vector.bn_aggr(out=mv, in_=stats)
                    msq = npool.tile([128, 1], F32, tag="msq")
                    nc.vector.tensor_mul(out=msq, in0=mv[:, 0:1], in1=mv[:, 0:1])
                    nc.vector.tensor_add(out=msq, in0=msq, in1=mv[:, 1:2])
                    rstd = npool.tile([128, 1], F32, tag="rstd")
                    nc.scalar.activation(out=rstd, in_=msq, func=AF.Sqrt, bias=eps_sb)
                    nc.vector.reciprocal(out=rstd, in_=rstd)
                    h_t = npool.tile([128, D], BF16, tag="h_t")
                    nc.vector.tensor_scalar_mul(out=h_t, in0=xsl, scalar1=rstd)
                    # transpose on PE (idle here): h_t [128 t, D] -> hT chunks
                    for dch in range(8):
                        ps_t = nps.tile([128, 128], BF16, tag="ps_t")
                        nc.tensor.transpose(out=ps_t, in_=h_t[:, ts(dch, 128)],
                                            identity=ident_sb)
                        nc.vector.tensor_copy(out=hT_own[:, dch, ts(tt4, 128)],
                                              in_=ps_t)
                nc.sync.dma_start(
                    out=tin_dram[:].rearrange("(c p) t -> p c t", p=128), in_=hT_own)
            nc.gpsimd.collective_compute(
                "AllGather", mybir.AluOpType.bypass, replica_groups=rg,
                ins=[tin_dram[:]], outs=[ag_out_dram[:]])

        def load_hT(ag_dram):
            hT = big.tile([128, 8, B * S], BF16, tag="big")
            for j in range(N):
                nc.sync.dma_start(
                    out=hT[:, :, ts(j, T)],
                    in_=ag_dram[ts(j, D), :].rearrange("(c p) t -> p c t", p=128))
            return hT

        # ================= phase 1: norm1 + AG =================
        rmsnorm_to_ag(xm_sb, h1_tm, h1t_in, h1t_ag)

        # weight/const loads land here: they execute during the AllGather
        trigC_sb = consts.tile([128, S], F32)
        nc.sync.dma_start(out=trigC_sb, in_=trigC[:])
        trigS_sb = consts.tile([128, S], F32)
        nc.sync.dma_start(out=trigS_sb, in_=trigS[:])
        mask_sb = consts.tile([128, 128], BF16)
        nc.sync.dma_start(out=mask_sb, in_=mask_in[:])
        perm_sb = consts.tile([128, 128], BF16)
        nc.sync.dma_start(out=perm_sb, in_=perm_in[:])
        wqT_sb = load_w("wqT", wqT, 128)
        wkT_sb = load_w("wkT", wkT, 128)
        wvT_sb = load_w("wvT", wvT, 128)
        woT_sb = load_w("woT", woT, D)
        w1T_sb = load_w("w1T", w1T, DFFC)
        w3T_sb = load_w("w3T", w3T, DFFC)

        hT_sb = load_hT(h1t_ag)

        # ================= phase 2: QKV + RoPE =================
        with (
            tc.tile_pool(name="qkv_ps", bufs=2, space="PSUM") as qkv_ps,
            tc.tile_pool(name="rope", bufs=2) as rope,
        ):
            for tt in range(8):
                pos = (tt % 4) * 512
                for dst_sb, wT_t in ((qT_sb, wqT_sb), (kT_sb, wkT_sb)):
                    ps_x = qkv_ps.tile([128, 512], F32, tag="psx")
                    for dch in range(8):
                        nc.tensor.matmul(out=ps_x, lhsT=wT_t[:, dch, :],
                                         rhs=hT_sb[:, dch, ts(tt, 512)],
                                         start=dch == 0, stop=dch == 7)
                    # swapped-rows copy via permutation matmul (E<->O halves)
                    x_bf = rope.tile([128, 512], BF16, tag="x_bf")
                    nc.vector.tensor_copy(out=x_bf, in_=ps_x)
                    ps_xs = qkv_ps.tile([128, 512], F32, tag="psxs")
                    nc.tensor.matmul(out=ps_xs, lhsT=perm_sb, rhs=x_bf,
                                     start=True, stop=True)
                    t1 = rope.tile([128, 512], F32, tag="r1")
                    nc.vector.tensor_mul(out=t1, in0=ps_x,
                                         in1=trigC_sb[:, ds(pos, 512)])
                    t2 = rope.tile([128, 512], F32, tag="r2")
                    nc.vector.tensor_mul(out=t2, in0=ps_xs,
                                         in1=trigS_sb[:, ds(pos, 512)])
                    nc.vector.tensor_add(out=dst_sb[:, ts(tt, 512)], in0=t1, in1=t2)
                for st in range(4):
                    tg = tt * 4 + st
                    ps_v = qkv_ps.tile([128, 128], F32, tag="psv")
                    for dch in range(8):
                        nc.tensor.matmul(out=ps_v,
                                         lhsT=hT_sb[:, dch, ds(tt * 512 + st * 128, 128)],
                                         rhs=wvT_sb[:, dch, :],
                                         start=dch == 0, stop=dch == 7)
                    nc.vector.tensor_copy(out=v_sb[:, tg, 0:64], in_=ps_v[:, 0:64])
                    nc.vector.tensor_copy(out=v_sb[:, tg, 65:129], in_=ps_v[:, 64:128])
            nc.vector.memset(v_sb[:, :, 64:65], 1.0)
            nc.vector.memset(v_sb[:, :, 129:130], 1.0)

        # ================= phase 3: attention =================
        with (
            tc.tile_pool(name="attn_ps", bufs=2, space="PSUM") as attn_ps,
            tc.tile_pool(name="attn_sb", bufs=3) as attn_sb,
        ):
            for b in range(B):
                for h in range(HPC):
                    fr = 64 * h
                    vcol = 65 * h
                    for qt in range(4):
                        qbase = b * S + qt * 512
                        ps_o = attn_ps.tile([65, 512], F32, tag="ps_o")
                        nkt = 4 * qt + 4
                        for kt in range(nkt):
                            d_off = kt * 128 - qt * 512
                            c0 = max(d_off, 0)
                            ps_s = attn_ps.tile([128, 512], F32, tag="ps_s")
                            nc.tensor.matmul(
                                out=ps_s[:, c0:512],
                                lhsT=kT_sb[fr:fr + 64, ds(b * S + kt * 128, 128)],
                                rhs=qT_sb[fr:fr + 64, ds(qbase + c0, 512 - c0)],
                                start=True, stop=True)
                            pT = attn_sb.tile([128, 512], BF16, tag="pT")
                            nc.scalar.activation(out=pT[:, c0:512], in_=ps_s[:, c0:512],
                                                 func=AF.Exp)
                            if d_off >= 0:
                                nc.vector.tensor_mul(
                                    out=pT[:, ds(d_off, 128)],
                                    in0=pT[:, ds(d_off, 128)], in1=mask_sb)
                            nc.tensor.matmul(
                                out=ps_o[:, c0:512],
                                lhsT=v_sb[:, b * 16 + kt, vcol:vcol + 65],
                                rhs=pT[:, c0:512],
                                start=kt == 0, stop=kt == nkt - 1)
                        rec = attn_sb.tile([1, 512], F32, tag="rec")
                        nc.vector.reciprocal(out=rec, in_=ps_o[64:65, :])
                        rec_bf = attn_sb.tile([1, 512], BF16, tag="rec_bf")
                        nc.vector.tensor_copy(out=rec_bf, in_=rec)
                        ps_b = attn_ps.tile([64, 512], F32, tag="ps_b")
                        nc.tensor.matmul(out=ps_b, lhsT=ones_sb, rhs=rec_bf,
                                         start=True, stop=True)
                        bc_sb = attn_sb.tile([64, 512], F32, tag="bc")
                        nc.vector.tensor_copy(out=bc_sb, in_=ps_b)
                        nc.vector.tensor_mul(out=oT_sb[fr:fr + 64, ds(qbase, 512)],
                                             in0=ps_o[0:64, :], in1=bc_sb)

        # ================= phase 4: A2A of head outputs =================
        for j in range(N):
            nc.sync.dma_start(out=o_a2a_in[ts(j, 128), :], in_=oT_sb[:, ts(j, T)])
        nc.gpsimd.collective_compute(
            "AllToAll", mybir.AluOpType.bypass, replica_groups=rg,
            ins=[o_a2a_in[:]], outs=[o_a2a_out[:]])
        oag_sb = persist.tile([128, 8, T], BF16)
        nc.sync.dma_start(out=oag_sb,
                          in_=o_a2a_out[:].rearrange("(c p) t -> p c t", p=128))

        # ================= phase 5: o-proj + residual =================
        with tc.tile_pool(name="op_ps", bufs=2, space="PSUM") as op_ps:
            for tc4 in range(4):
                for n in range(2):
                    ps_op = op_ps.tile([128, 512], F32, tag="ps_op")
                    for fch in range(8):
                        nc.tensor.matmul(out=ps_op,
                                         lhsT=oag_sb[:, fch, ts(tc4, 128)],
                                         rhs=woT_sb[:, fch, ts(n, 512)],
                                         start=fch == 0, stop=fch == 7)
                    nc.vector.tensor_add(out=xm_sb[:, tc4, ts(n, 512)],
                                         in0=xm_sb[:, tc4, ts(n, 512)], in1=ps_op)

        # ================= phase 6: norm2 + AG =================
        rmsnorm_to_ag(xm_sb, h2_tm, h2t_in, h2t_ag)
        hT2_sb = load_hT(h2t_ag)

        # ================= phase 7: FFN up + SwiGLU =================
        # dff-outer so the first half of s finishes early and its AllToAll
        # overlaps the second half's compute.
        with (
            tc.tile_pool(name="ffn_ps", bufs=2, space="PSUM") as ffn_ps,
            tc.tile_pool(name="ffn_sb", bufs=3) as ffn_sb,
        ):
            for dc in range(4):
                for tt in range(8):
                    ps_u = ffn_ps.tile([128, 512], F32, tag="ps_u")
                    for dch in range(8):
                        nc.tensor.matmul(out=ps_u,
                                         lhsT=w1T_sb[:, dch, ts(dc, 128)],
                                         rhs=hT2_sb[:, dch, ts(tt, 512)],
                                         start=dch == 0, stop=dch == 7)
                    ps_g = ffn_ps.tile([128, 512], F32, tag="ps_g")
                    for dch in range(8):
                        nc.tensor.matmul(out=ps_g,
                                         lhsT=w3T_sb[:, dch, ts(dc, 128)],
                                         rhs=hT2_sb[:, dch, ts(tt, 512)],
                                         start=dch == 0, stop=dch == 7)
                    silu_t = ffn_sb.tile([128, 512], F32, tag="silu")
                    nc.scalar.activation(out=silu_t, in_=ps_u, func=AF.Silu)
                    s_t = ffn_sb.tile([128, 512], BF16, tag="s_t")
                    nc.vector.tensor_mul(out=s_t, in0=silu_t, in1=ps_g)
                    s_in = s_a2a_in_a if dc < 2 else s_a2a_in_b
                    nc.sync.dma_start(
                        out=s_in[ds(tt * 256 + (dc % 2) * 128, 128), :], in_=s_t)
                if dc == 1:
                    nc.gpsimd.collective_compute(
                        "AllToAll", mybir.AluOpType.bypass, replica_groups=rg,
                        ins=[s_a2a_in_a[:]], outs=[s_a2a_out_a[:]])
        nc.gpsimd.collective_compute(
            "AllToAll", mybir.AluOpType.bypass, replica_groups=rg,
            ins=[s_a2a_in_b[:]], outs=[s_a2a_out_b[:]])

        # ================= phase 8: down-proj + residual =================
        # sT chunk order: a-half chunks (j*2+c2) then b-half; w2T rows are
        # host-permuted to match.
        sT_sb = big.tile([128, 32, T], BF16, tag="big")
        nc.sync.dma_start(out=sT_sb[:, 0:16, :],
                          in_=s_a2a_out_a[:].rearrange("(c p) t -> p c t", p=128))
        nc.sync.dma_start(out=sT_sb[:, 16:32, :],
                          in_=s_a2a_out_b[:].rearrange("(c p) t -> p c t", p=128))
        with (
            tc.tile_pool(name="dn_ps", bufs=1, space="PSUM") as dn_ps,
            tc.tile_pool(name="dn_sb", bufs=8) as dn_sb,
        ):
            ps_d = [dn_ps.tile([128, 512], F32, tag=f"ps_d{i}", name=f"ps_d{i}")
                    for i in range(8)]
            for dc in range(32):
                for n in range(2):
                    w2c = dn_sb.tile([128, 512], BF16, tag=f"w2c{n}", name="w2c")
                    nc.scalar.dma_start(out=w2c, in_=w2T[ts(dc, 128), ts(n, 512)])
                    for tc4 in range(4):
                        nc.tensor.matmul(out=ps_d[n * 4 + tc4],
                                         lhsT=sT_sb[:, dc, ts(tc4, 128)],
                                         rhs=w2c,
                                         start=dc == 0, stop=dc == 31)
            for n in range(2):
                for tc4 in range(4):
                    o_t = dn_sb.tile([128, 512], F32, tag="o_t")
                    nc.vector.tensor_add(out=o_t, in0=xm_sb[:, tc4, ts(n, 512)],
                                         in1=ps_d[n * 4 + tc4])
                    nc.sync.dma_start(
                        out=out_ext[:].rearrange("(t p) d -> p t d", p=128)[:, tc4, ts(n, 512)],
                        in_=o_t)

    nc.compile()
    return nc


def _host_prep(inputs):
    x = np.asarray(inputs["x"], np.float32).reshape(B * S, D)
    w_q = np.asarray(inputs["w_q"], np.float32)
    w_k = np.asarray(inputs["w_k"], np.float32)
    w_v = np.asarray(inputs["w_v"], np.float32)
    w_o = np.asarray(inputs["w_o"], np.float32)
    ln1 = np.asarray(inputs["ln1_w"], np.float32)
    ln2 = np.asarray(inputs["ln2_w"], np.float32)
    w1 = np.asarray(inputs["w1"], np.float32)
    w2 = np.asarray(inputs["w2"], np.float32)
    w3 = np.asarray(inputs["w3"], np.float32)

    wq_f = (w_q * ln1[None, :]) / np.sqrt(DK)
    wk_f = w_k * ln1[None, :]
    wv_f = w_v * ln1[None, :]
    w1_f = w1 * ln2[None, :]
    w3_f = w3 * ln2[None, :]

    # RoPE feature permutation: per core rows [h0E, h0O, h1E, h1O]
    jj = np.arange(32)
    swap_rows = np.concatenate([jj + 32, jj, jj + 96, jj + 64])
    # perm matmul matrix: out[m] = in[swap_rows[m]] -> P[k, m] = 1 iff k = swap(m)
    perm_mat = np.zeros((128, 128), dtype=BF16NP)
    perm_mat[swap_rows, np.arange(128)] = 1.0

    inv_freq = THETA ** (-(np.arange(0, DK, 2, dtype=np.float32) / DK))
    t_pos = np.arange(S, dtype=np.float32)
    ang = inv_freq[:, None] * t_pos[None, :]          # [32, S]
    c32, s32 = np.cos(ang), np.sin(ang)
    trigC = np.concatenate([c32, c32, c32, c32]).astype(np.float32)
    trigS = np.concatenate([-s32, s32, -s32, s32]).astype(np.float32)

    ident = np.eye(128, dtype=BF16NP)
    k_idx = np.arange(128)[:, None]
    q_idx = np.arange(128)[None, :]
    mask = (q_idx >= k_idx).astype(BF16NP)

    woT = np.ascontiguousarray(w_o.T).astype(BF16NP)
    # w2T rows ordered to match the consumer's split-A2A chunk order:
    # a-half (j, c2) -> global rows j*512 + c2*128, then b-half (+256)
    row_order = []
    for half in range(2):
        for j in range(N):
            for c2 in range(2):
                base = j * DFFC + half * 256 + c2 * 128
                row_order.extend(range(base, base + 128))
    w2T = np.ascontiguousarray(w2.T[np.array(row_order)]).astype(BF16NP)

    in_maps = []
    for i in range(N):
        perm = []
        for h in range(HPC):
            base = (HPC * i + h) * DK
            perm.extend(base + 2 * jj)       # even
            perm.extend(base + 2 * jj + 1)   # odd
        perm = np.array(perm)
        wq_p = wq_f[perm]                    # [128, 1024]
        wk_p = wk_f[perm]
        wqT = np.ascontiguousarray(wq_p.T).astype(BF16NP)
        wkT = np.ascontiguousarray(wk_p.T).astype(BF16NP)
        in_maps.append({
            "x": np.ascontiguousarray(x[i * T:(i + 1) * T]),
            "wqT": wqT,
            "wkT": wkT,
            "rope_perm": perm_mat,
            "ident": ident,
            "wvT": np.ascontiguousarray(wv_f[i * 128:(i + 1) * 128].T).astype(BF16NP),
            "woT": woT,
            "w1T": np.ascontiguousarray(w1_f[i * DFFC:(i + 1) * DFFC].T).astype(BF16NP),
            "w3T": np.ascontiguousarray(w3_f[i * DFFC:(i + 1) * DFFC].T).astype(BF16NP),
            "w2T": w2T,
            "trigC": trigC,
            "trigS": trigS,
            "mask": mask,
        })
    return in_maps


def _get_nc():
    global _NC
    if _NC is None:
        _NC = _build()
    return _NC


def run(inputs, trace=False):
    nc = _get_nc()
    in_maps = _host_prep(inputs)
    res = run_bass_kernel_spmd(nc, in_maps, list(range(N)), trace=trace)
    out = np.concatenate([res.results[i]["out"] for i in range(N)], axis=0)
    return out.reshape(B, S, D).astype(np.float32), res


def kernel(**inputs):
    out, _ = run(inputs)
    return out



# revision 21
# speedup vs baseline: 1.1273x; 1.1273x over previous
"""Trainium2 8-core tensor-parallel transformer block (RMSNorm + RoPE causal
attention + SwiGLU FFN).

Sharding (SPMD, identical program on 8 cores, per-core data via in_maps):
  - attention: heads sharded (2 heads/core); q/k/v projections row-sharded;
    normalized activations AllGathered (bf16, 2 chunks for load overlap)
  - o-projection: token-sharded after an AllToAll of head outputs
  - FFN: token-sharded — each core runs the FULL dff=4096 SwiGLU for its own
    512 tokens, streaming full w1/w3/w2 from HBM (no second AllGather and no
    activation AllToAll; weight streaming hides under the matmuls)

All matmuls bf16 x bf16 -> fp32 PSUM. Residual spine fp32.
RoPE is applied as q_rot = X*C + X_swap*S where X_swap comes from a second
matmul with row-swapped weights; q/k rows are host-permuted to
[h0-even, h0-odd, h1-even, h1-odd], which leaves q.k dot products unchanged.
Softmax skips the running max (|scores| < 4 for this problem's scale) and
gets its denominator for free from a ones-column appended to V.
Score matmuls for the two heads are row-tiled (K=64 each at array rows 0-63 /
64-127) so they run concurrently on the PE array; exp is done for both heads
in one ACT op; denominators use the fast DVE reciprocal and one broadcast
matmul per (batch, q-tile).
"""

from contextlib import ExitStack

import numpy as np
import ml_dtypes

import concourse.mybir as mybir
import concourse.tile as tile
from concourse import bacc
from concourse.bass import ds, ts
from concourse.bass_utils import run_bass_kernel_spmd

B, S, D, H, DK, DFF = 2, 2048, 1024, 16, 64, 4096
THETA, EPS = 10000.0, 1e-5
N = 8            # cores
T = B * S // N   # tokens per core (512)
HPC = H // N     # heads per core (2)

F32 = mybir.dt.float32
BF16 = mybir.dt.bfloat16
BF16NP = ml_dtypes.bfloat16
AF = mybir.ActivationFunctionType

_NC = None


def _build():
    nc = bacc.Bacc("TRN2", target_bir_lowering=False)

    # ---- I/O ----
    x_in = nc.dram_tensor("x", [T, D], F32, kind="ExternalInput")
    wqT = nc.dram_tensor("wqT", [D, 128], BF16, kind="ExternalInput")
    wkT = nc.dram_tensor("wkT", [D, 128], BF16, kind="ExternalInput")
    perm_in = nc.dram_tensor("rope_perm", [128, 128], BF16, kind="ExternalInput")
    ident_in = nc.dram_tensor("ident", [128, 128], BF16, kind="ExternalInput")
    wvT = nc.dram_tensor("wvT", [D, 128], BF16, kind="ExternalInput")
    woT = nc.dram_tensor("woT", [D, D], BF16, kind="ExternalInput")
    w1T = nc.dram_tensor("w1T", [D, DFF], BF16, kind="ExternalInput")
    w3T = nc.dram_tensor("w3T", [D, DFF], BF16, kind="ExternalInput")
    w2T = nc.dram_tensor("w2T", [DFF, D], BF16, kind="ExternalInput")
    trigC = nc.dram_tensor("trigC", [128, S], F32, kind="ExternalInput")
    trigS = nc.dram_tensor("trigS", [128, S], F32, kind="ExternalInput")
    mask_in = nc.dram_tensor("mask", [128, 128], BF16, kind="ExternalInput")
    out_ext = nc.dram_tensor("out", [T, D], F32, kind="ExternalOutput")

    # ---- internal DRAM ----
    # norm1 output, transposed, split in two D-chunks for AG/load overlap
    h1t_in = [nc.dram_tensor(f"h1t_in{c}", [D // 2, T], BF16) for c in range(2)]
    h1t_ag = [nc.dram_tensor(f"h1t_ag{c}", [N * D // 2, T], BF16,
                             addr_space="Shared") for c in range(2)]
    o_a2a_in = nc.dram_tensor("o_a2a_in", [N * 128, T], BF16)
    o_a2a_out = nc.dram_tensor("o_a2a_out", [N * 128, T], BF16)

    rg = [list(range(N))]

    with tile.TileContext(nc) as tc, ExitStack() as stack:
        consts = stack.enter_context(tc.tile_pool(name="consts", bufs=1))
        persist = stack.enter_context(tc.tile_pool(name="persist", bufs=1))
        wpool = stack.enter_context(tc.tile_pool(name="wpool", bufs=1))

        # x first — it heads the critical path into norm1 + AG; chunked so
        # stats start on the first 128-token group immediately
        xm_sb = persist.tile([128, 4, D], F32)   # x, later mid (x + attn)
        for tt4 in range(4):
            nc.sync.dma_start(
                out=xm_sb[:, tt4, :],
                in_=x_in[:].rearrange("(t p) d -> p t d", p=128)[:, tt4, :])

        ones_sb = consts.tile([1, 64], BF16)
        nc.vector.memset(ones_sb, 1.0)
        ident_sb = consts.tile([128, 128], BF16)
        nc.sync.dma_start(out=ident_sb, in_=ident_in[:])
        eps_sb = consts.tile([128, 1], F32)
        nc.vector.memset(eps_sb, EPS)

        # ---- norm helper: src [128,4,D] f32 -> hT tile [128,8,T] bf16 ----
        # transposes ordered dch-pair-major so D-chunks finish early (AG overlap)
        def rmsnorm_transpose(src_sb, npool, nps, hT_own, chunk_done=None):
            hts = []
            for tt4 in range(4):
                xsl = src_sb[:, tt4, :]
                stats = npool.tile([128, 2, 6], F32, tag="stats")
                nc.vector.bn_stats(out=stats[:, 0, :], in_=xsl[:, 0:512])
                nc.vector.bn_stats(out=stats[:, 1, :], in_=xsl[:, 512:1024])
                mv = npool.tile([128, 2], F32, tag="mv")
                nc.vector.bn_aggr(out=mv, in_=stats)
                msq = npool.tile([128, 1], F32, tag="msq")
                nc.vector.tensor_mul(out=msq, in0=mv[:, 0:1], in1=mv[:, 0:1])
                nc.vector.tensor_add(out=msq, in0=msq, in1=mv[:, 1:2])
                rstd = npool.tile([128, 1], F32, tag="rstd")
                nc.scalar.activation(out=rstd, in_=msq, func=AF.Sqrt, bias=eps_sb)
                nc.vector.reciprocal(out=rstd, in_=rstd)
                h_t = npool.tile([128, D], BF16, tag=f"h_t{tt4}")
                nc.vector.tensor_scalar_mul(out=h_t, in0=xsl, scalar1=rstd)
                hts.append(h_t)
            for c in range(2):
                for dch in range(4 * c, 4 * c + 4):
                    for tt4 in range(4):
                        ps_t = nps.tile([128, 128], BF16, tag="ps_t")
                        nc.tensor.transpose(out=ps_t, in_=hts[tt4][:, ts(dch, 128)],
                                            identity=ident_sb)
                        nc.vector.tensor_copy(out=hT_own[:, dch, ts(tt4, 128)],
                                              in_=ps_t)
                if chunk_done is not None:
                    chunk_done(c, hT_own)

        qT_sb = persist.tile([128, B * S], BF16)
        kT_sb = persist.tile([128, B * S], BF16)
        v_sb = persist.tile([128, 32, 130], BF16)
        oT_sb = persist.tile([128, B * S], BF16)

        # ================= phase 1: norm1 + chunked AG =================
        def ag_chunk(c, hT_own):
            nc.sync.dma_start(
                out=h1t_in[c][:].rearrange("(c2 p) t -> p c2 t", p=128),
                in_=hT_own[:, 4 * c:4 * c + 4, :])
            nc.gpsimd.collective_compute(
                "AllGather", mybir.AluOpType.bypass, replica_groups=rg,
                ins=[h1t_in[c][:]], outs=[h1t_ag[c][:]])

        with (
            tc.tile_pool(name="norm1", bufs=2) as npool1,
            tc.tile_pool(name="norm1_ps", bufs=2, space="PSUM") as nps1,
        ):
            h1T_own = npool1.tile([128, 8, T], BF16, tag="hT_own")
            rmsnorm_transpose(xm_sb, npool1, nps1, h1T_own, chunk_done=ag_chunk)

        # weight/const loads land here: they execute during the AllGather
        trigC_sb = consts.tile([128, S], F32)
        nc.sync.dma_start(out=trigC_sb, in_=trigC[:])
        trigS_sb = consts.tile([128, S], F32)
        nc.sync.dma_start(out=trigS_sb, in_=trigS[:])
        mask_sb = consts.tile([128, 128], BF16)
        nc.sync.dma_start(out=mask_sb, in_=mask_in[:])
        perm_sb = consts.tile([128, 128], BF16)
        nc.sync.dma_start(out=perm_sb, in_=perm_in[:])

        def load_w(name, dram, cols):
            t = wpool.tile([128, 8, cols], BF16, tag=name)
            nc.scalar.dma_start(out=t, in_=dram[:].rearrange("(c p) f -> p c f", p=128))
            return t

        wqT_sb = load_w("wqT", wqT, 128)
        wkT_sb = load_w("wkT", wkT, 128)
        wvT_sb = load_w("wvT", wvT, 128)
        woT_sb = load_w("woT", woT, D)

        with tc.tile_pool(name="big", bufs=1) as big:
            hT_sb = big.tile([128, 8, B * S], BF16, tag="big")
            for c in range(2):
                for j in range(N):
                    nc.sync.dma_start(
                        out=hT_sb[:, 4 * c:4 * c + 4, ts(j, T)],
                        in_=h1t_ag[c][ts(j, D // 2), :].rearrange(
                            "(c2 p) t -> p c2 t", p=128))

            # ================= phase 2: QKV + RoPE =================
            with (
                tc.tile_pool(name="qkv_ps", bufs=2, space="PSUM") as qkv_ps,
                tc.tile_pool(name="rope", bufs=2) as rope,
            ):
                for tt in range(8):
                    pos = (tt % 4) * 512
                    for dst_sb, wT_t in ((qT_sb, wqT_sb), (kT_sb, wkT_sb)):
                        ps_x = qkv_ps.tile([128, 512], F32, tag="psx")
                        for dch in range(8):
                            nc.tensor.matmul(out=ps_x, lhsT=wT_t[:, dch, :],
                                             rhs=hT_sb[:, dch, ts(tt, 512)],
                                             start=dch == 0, stop=dch == 7)
                        # swapped-rows copy via permutation matmul (E<->O halves)
                        x_bf = rope.tile([128, 512], BF16, tag="x_bf")
                        nc.vector.tensor_copy(out=x_bf, in_=ps_x)
                        ps_xs = qkv_ps.tile([128, 512], F32, tag="psxs")
                        nc.tensor.matmul(out=ps_xs, lhsT=perm_sb, rhs=x_bf,
                                         start=True, stop=True)
                        t1 = rope.tile([128, 512], F32, tag="r1")
                        nc.vector.tensor_mul(out=t1, in0=ps_x,
                                             in1=trigC_sb[:, ds(pos, 512)])
                        t2 = rope.tile([128, 512], F32, tag="r2")
                        nc.vector.tensor_mul(out=t2, in0=ps_xs,
                                             in1=trigS_sb[:, ds(pos, 512)])
                        nc.vector.tensor_add(out=dst_sb[:, ts(tt, 512)],
                                             in0=t1, in1=t2)
                    for st in range(4):
                        tg = tt * 4 + st
                        ps_v = qkv_ps.tile([128, 128], F32, tag="psv")
                        for dch in range(8):
                            nc.tensor.matmul(
                                out=ps_v,
                                lhsT=hT_sb[:, dch, ds(tt * 512 + st * 128, 128)],
                                rhs=wvT_sb[:, dch, :],
                                start=dch == 0, stop=dch == 7)
                        nc.vector.tensor_copy(out=v_sb[:, tg, 0:64], in_=ps_v[:, 0:64])
                        nc.vector.tensor_copy(out=v_sb[:, tg, 65:129], in_=ps_v[:, 64:128])
                nc.vector.memset(v_sb[:, :, 64:65], 1.0)
                nc.vector.memset(v_sb[:, :, 129:130], 1.0)

        # ================= phase 3: attention =================
        # score matmuls packed: head0 at array rows 0-63, head1 at rows 64-127
        # (concurrent row-tiled MMs); exp over both heads in one ACT op.
        with (
            tc.tile_pool(name="pair_ps", bufs=2, space="PSUM") as pair_ps,
            tc.tile_pool(name="psum_o", bufs=2, space="PSUM") as psum_o,
            tc.tile_pool(name="attn_sb", bufs=3) as attn_sb,
        ):
            for b in range(B):
                for qt in range(4):
                    qbase = b * S + qt * 512
                    ps_o = [psum_o.tile([65, 512], F32, tag=f"ps_o{h}",
                                        name=f"ps_o{h}")
                            for h in range(2)]
                    nkt = 4 * qt + 4
                    for kt in range(nkt):
                        d_off = kt * 128 - qt * 512
                        c0 = max(d_off, 0)
                        pair = pair_ps.tile([128, 2, 512], F32, tag="pair")
                        for h in range(2):
                            nc.tensor.matmul(
                                out=pair[:, h, c0:512],
                                lhsT=kT_sb[64 * h:64 * h + 64,
                                           ds(b * S + kt * 128, 128)],
                                rhs=qT_sb[64 * h:64 * h + 64, ds(qbase + c0, 512 - c0)],
                                start=True, stop=True)
                        pT = attn_sb.tile([128, 2, 512], BF16, tag="pT")
                        nc.scalar.activation(out=pT[:, :, c0:512],
                                             in_=pair[:, :, c0:512], func=AF.Exp)
                        if d_off >= 0:
                            for h in range(2):
                                nc.vector.tensor_mul(
                                    out=pT[:, h, ds(d_off, 128)],
                                    in0=pT[:, h, ds(d_off, 128)], in1=mask_sb)
                        for h in range(2):
                            nc.tensor.matmul(
                                out=ps_o[h][:, c0:512],
                                lhsT=v_sb[:, b * 16 + kt, 65 * h:65 * h + 65],
                                rhs=pT[:, h, c0:512],
                                start=kt == 0, stop=kt == nkt - 1)
                    # softmax denominators: fast reciprocal + col-tiled
                    # broadcast MMs (h0 -> psum partitions 0-63, h1 -> 64-127)
                    bc = pair_ps.tile([128, 2, 512], F32, tag="pair")
                    for h in range(2):
                        # reciprocal_approx_fast (custom DVE op) misreads APs
                        # whose base partition is 64 — stage the denominator
                        # row into a base-0 SBUF tile first.
                        den_f = attn_sb.tile([1, 512], F32, tag=f"den_f{h}",
                                             name="den_f")
                        nc.vector.tensor_copy(out=den_f, in_=ps_o[h][64:65, :])
                        rec_f = attn_sb.tile([1, 512], F32, tag=f"rec_f{h}",
                                             name="rec_f")
                        nc.vector.reciprocal_approx_fast(out=rec_f, in_=den_f)
                        rec_b = attn_sb.tile([1, 512], BF16, tag=f"rec_b{h}",
                                             name="rec_b")
                        nc.vector.tensor_copy(out=rec_b, in_=rec_f)
                        nc.tensor.matmul(out=bc[64 * h:64 * h + 64, 0, :],
                                         lhsT=ones_sb, rhs=rec_b,
                                         start=True, stop=True)
                    bc_sb = attn_sb.tile([128, 512], F32, tag="bc_sb")
                    nc.vector.tensor_copy(out=bc_sb, in_=bc[:, 0, :])
                    for h in range(2):
                        nc.vector.tensor_mul(
                            out=oT_sb[64 * h:64 * h + 64, ds(qbase, 512)],
                            in0=ps_o[h][0:64, :], in1=bc_sb[64 * h:64 * h + 64, :])

        # ================= phase 4: A2A of head outputs =================
        for j in range(N):
            nc.sync.dma_start(out=o_a2a_in[ts(j, 128), :], in_=oT_sb[:, ts(j, T)])
        nc.gpsimd.collective_compute(
            "AllToAll", mybir.AluOpType.bypass, replica_groups=rg,
            ins=[o_a2a_in[:]], outs=[o_a2a_out[:]])
        oag_sb = persist.tile([128, 8, T], BF16)
        nc.sync.dma_start(out=oag_sb,
                          in_=o_a2a_out[:].rearrange("(c p) t -> p c t", p=128))

        # ================= phase 5: o-proj + residual =================
        with tc.tile_pool(name="op_ps", bufs=2, space="PSUM") as op_ps:
            for tc4 in range(4):
                for n in range(2):
                    ps_op = op_ps.tile([128, 512], F32, tag="ps_op")
                    for fch in range(8):
                        nc.tensor.matmul(out=ps_op,
                                         lhsT=oag_sb[:, fch, ts(tc4, 128)],
                                         rhs=woT_sb[:, fch, ts(n, 512)],
                                         start=fch == 0, stop=fch == 7)
                    nc.vector.tensor_add(out=xm_sb[:, tc4, ts(n, 512)],
                                         in0=xm_sb[:, tc4, ts(n, 512)], in1=ps_op)

        # ================= phase 6: norm2 (local only, token-sharded FFN) ====
        with tc.tile_pool(name="ffn", bufs=1) as ffn_pool:
            h2T_sb = ffn_pool.tile([128, 8, T], BF16, tag="h2T")
            with (
                tc.tile_pool(name="norm2", bufs=2) as npool2,
                tc.tile_pool(name="norm2_ps", bufs=2, space="PSUM") as nps2,
            ):
                rmsnorm_transpose(xm_sb, npool2, nps2, h2T_sb)

            # ============ phase 7: FFN up + SwiGLU (full dff, own tokens) ====
            sT_sb = ffn_pool.tile([128, 32, T], BF16, tag="sT")
            with (
                tc.tile_pool(name="ffn_ps", bufs=2, space="PSUM") as ffn_ps,
                tc.tile_pool(name="wstream", bufs=3) as wstream,
                tc.tile_pool(name="ffn_sb", bufs=3) as ffn_sb,
            ):
                for dc in range(32):
                    w1c = wstream.tile([128, 8, 128], BF16, tag="w1c")
                    nc.sync.dma_start(
                        out=w1c,
                        in_=w1T[:, ts(dc, 128)].rearrange("(c p) f -> p c f", p=128))
                    w3c = wstream.tile([128, 8, 128], BF16, tag="w3c")
                    nc.sync.dma_start(
                        out=w3c,
                        in_=w3T[:, ts(dc, 128)].rearrange("(c p) f -> p c f", p=128))
                    ps_u = ffn_ps.tile([128, 512], F32, tag="ps_u")
                    for dch in range(8):
                        nc.tensor.matmul(out=ps_u, lhsT=w1c[:, dch, :],
                                         rhs=h2T_sb[:, dch, :],
                                         start=dch == 0, stop=dch == 7)
                    ps_g = ffn_ps.tile([128, 512], F32, tag="ps_g")
                    for dch in range(8):
                        nc.tensor.matmul(out=ps_g, lhsT=w3c[:, dch, :],
                                         rhs=h2T_sb[:, dch, :],
                                         start=dch == 0, stop=dch == 7)
                    silu_t = ffn_sb.tile([128, 512], F32, tag="silu")
                    nc.scalar.activation(out=silu_t, in_=ps_u, func=AF.Silu)
                    nc.vector.tensor_mul(out=sT_sb[:, dc, :], in0=silu_t, in1=ps_g)

            # ================= phase 8: down-proj + residual =================
            with (
                tc.tile_pool(name="dn_ps", bufs=1, space="PSUM") as dn_ps,
                tc.tile_pool(name="dn_sb", bufs=8) as dn_sb,
            ):
                ps_d = [dn_ps.tile([128, 512], F32, tag=f"ps_d{i}", name=f"ps_d{i}")
                        for i in range(8)]
                for dc in range(32):
                    for n in range(2):
                        w2c = dn_sb.tile([128, 512], BF16, tag=f"w2c{n}", name="w2c")
                        nc.scalar.dma_start(out=w2c, in_=w2T[ts(dc, 128), ts(n, 512)])
                        for tc4 in range(4):
                            nc.tensor.matmul(out=ps_d[n * 4 + tc4],
                                             lhsT=sT_sb[:, dc, ts(tc4, 128)],
                                             rhs=w2c,
                                             start=dc == 0, stop=dc == 31)
                for n in range(2):
                    for tc4 in range(4):
                        o_t = dn_sb.tile([128, 512], F32, tag="o_t")
                        nc.vector.tensor_add(out=o_t, in0=xm_sb[:, tc4, ts(n, 512)],
                                             in1=ps_d[n * 4 + tc4])
                        nc.sync.dma_start(
                            out=out_ext[:].rearrange("(t p) d -> p t d", p=128)[:, tc4, ts(n, 512)],
                            in_=o_t)

    nc.compile()
    return nc


def _host_prep(inputs):
    x = np.asarray(inputs["x"], np.float32).reshape(B * S, D)
    w_q = np.asarray(inputs["w_q"], np.float32)
    w_k = np.asarray(inputs["w_k"], np.float32)
    w_v = np.asarray(inputs["w_v"], np.float32)
    w_o = np.asarray(inputs["w_o"], np.float32)
    ln1 = np.asarray(inputs["ln1_w"], np.float32)
    ln2 = np.asarray(inputs["ln2_w"], np.float32)
    w1 = np.asarray(inputs["w1"], np.float32)
    w2 = np.asarray(inputs["w2"], np.float32)
    w3 = np.asarray(inputs["w3"], np.float32)

    wq_f = (w_q * ln1[None, :]) / np.sqrt(DK)
    wk_f = w_k * ln1[None, :]
    wv_f = w_v * ln1[None, :]
    w1_f = w1 * ln2[None, :]
    w3_f = w3 * ln2[None, :]

    # RoPE feature permutation: per core rows [h0E, h0O, h1E, h1O]
    jj = np.arange(32)
    swap_rows = np.concatenate([jj + 32, jj, jj + 96, jj + 64])
    # perm matmul matrix: out[m] = in[swap_rows[m]] -> P[k, m] = 1 iff k = swap(m)
    perm_mat = np.zeros((128, 128), dtype=BF16NP)
    perm_mat[swap_rows, np.arange(128)] = 1.0

    inv_freq = THETA ** (-(np.arange(0, DK, 2, dtype=np.float32) / DK))
    t_pos = np.arange(S, dtype=np.float32)
    ang = inv_freq[:, None] * t_pos[None, :]          # [32, S]
    c32, s32 = np.cos(ang), np.sin(ang)
    trigC = np.concatenate([c32, c32, c32, c32]).astype(np.float32)
    trigS = np.concatenate([-s32, s32, -s32, s32]).astype(np.float32)

    ident = np.eye(128, dtype=BF16NP)
    k_idx = np.arange(128)[:, None]
    q_idx = np.arange(128)[None, :]
    mask = (q_idx >= k_idx).astype(BF16NP)

    woT = np.ascontiguousarray(w_o.T).astype(BF16NP)
    w1T_full = np.ascontiguousarray(w1_f.T).astype(BF16NP)   # [D, DFF]
    w3T_full = np.ascontiguousarray(w3_f.T).astype(BF16NP)   # [D, DFF]
    w2T_full = np.ascontiguousarray(w2.T).astype(BF16NP)     # [DFF, D]

    in_maps = []
    for i in range(N):
        perm = []
        for h in range(HPC):
            base = (HPC * i + h) * DK
            perm.extend(base + 2 * jj)       # even
            perm.extend(base + 2 * jj + 1)   # odd
        perm = np.array(perm)
        wq_p = wq_f[perm]                    # [128, 1024]
        wk_p = wk_f[perm]
        wqT_i = np.ascontiguousarray(wq_p.T).astype(BF16NP)
        wkT_i = np.ascontiguousarray(wk_p.T).astype(BF16NP)
        in_maps.append({
            "x": np.ascontiguousarray(x[i * T:(i + 1) * T]),
            "wqT": wqT_i,
            "wkT": wkT_i,
            "rope_perm": perm_mat,
            "ident": ident,
            "wvT": np.ascontiguousarray(wv_f[i * 128:(i + 1) * 128].T).astype(BF16NP),
            "woT": woT,
            "w1T": w1T_full,
            "w3T": w3T_full,
            "w2T": w2T_full,
            "trigC": trigC,
            "trigS": trigS,
            "mask": mask,
        })
    return in_maps


def _get_nc():
    global _NC
    if _NC is None:
        _NC = _build()
    return _NC


def run(inputs, trace=False):
    nc = _get_nc()
    in_maps = _host_prep(inputs)
    res = run_bass_kernel_spmd(nc, in_maps, list(range(N)), trace=trace)
    out = np.concatenate([res.results[i]["out"] for i in range(N)], axis=0)
    return out.reshape(B, S, D).astype(np.float32), res


def kernel(**inputs):
    out, _ = run(inputs)
    return out


# revision 25
# speedup vs baseline: 1.2501x; 1.1089x over previous
"""Trainium2 8-core tensor-parallel transformer block (RMSNorm + RoPE causal
attention + SwiGLU FFN).

Sharding (SPMD, identical program on 8 cores, per-core data via in_maps):
  - attention: heads sharded (2 heads/core); q/k/v projections row-sharded;
    normalized activations AllGathered (bf16, 2 chunks for load overlap)
  - o-projection: token-sharded after an AllToAll of head outputs
  - FFN: token-sharded — each core runs the FULL dff=4096 SwiGLU for its own
    512 tokens, streaming full w1/w3/w2 from HBM (no second AllGather and no
    activation AllToAll; weight streaming hides under the matmuls)

All matmuls bf16 x bf16 -> fp32 PSUM. Residual spine fp32.
RoPE is applied as q_rot = X*C + X_swap*S where X_swap comes from a second
matmul with row-swapped weights; q/k rows are host-permuted to
[h0-even, h0-odd, h1-even, h1-odd], which leaves q.k dot products unchanged.
Softmax skips the running max (|scores| < 4 for this problem's scale) and
gets its denominator for free from a ones-column appended to V.
Score matmuls for the two heads are row-tiled (K=64 each at array rows 0-63 /
64-127) so they run concurrently on the PE array; exp is done for both heads
in one ACT op; denominators use the fast DVE reciprocal and one broadcast
matmul per (batch, q-tile).
"""

from contextlib import ExitStack

import numpy as np
import ml_dtypes

import concourse.mybir as mybir
import concourse.tile as tile
from concourse import bacc
from concourse.bass import ds, ts
from concourse.bass_utils import run_bass_kernel_spmd

B, S, D, H, DK, DFF = 2, 2048, 1024, 16, 64, 4096
THETA, EPS = 10000.0, 1e-5
N = 8            # cores
T = B * S // N   # tokens per core (512)
HPC = H // N     # heads per core (2)

F32 = mybir.dt.float32
BF16 = mybir.dt.bfloat16
BF16NP = ml_dtypes.bfloat16
AF = mybir.ActivationFunctionType

_NC = None


def _build():
    nc = bacc.Bacc("TRN2", target_bir_lowering=False)

    # ---- I/O ----
    x_in = nc.dram_tensor("x", [T, D], F32, kind="ExternalInput")
    wqT = nc.dram_tensor("wqT", [D, 128], BF16, kind="ExternalInput")
    wkT = nc.dram_tensor("wkT", [D, 128], BF16, kind="ExternalInput")
    perm_in = nc.dram_tensor("rope_perm", [128, 128], BF16, kind="ExternalInput")
    ident_in = nc.dram_tensor("ident", [128, 128], BF16, kind="ExternalInput")
    wvT = nc.dram_tensor("wvT", [D, 128], BF16, kind="ExternalInput")
    woT = nc.dram_tensor("woT", [D, D], BF16, kind="ExternalInput")
    w1T = nc.dram_tensor("w1T", [D, DFF], BF16, kind="ExternalInput")
    w3T = nc.dram_tensor("w3T", [D, DFF], BF16, kind="ExternalInput")
    w2T = nc.dram_tensor("w2T", [DFF, D], BF16, kind="ExternalInput")
    trigC = nc.dram_tensor("trigC", [128, S], F32, kind="ExternalInput")
    trigS = nc.dram_tensor("trigS", [128, S], F32, kind="ExternalInput")
    mask_in = nc.dram_tensor("mask", [128, 128], BF16, kind="ExternalInput")
    out_ext = nc.dram_tensor("out", [T, D], F32, kind="ExternalOutput")

    # ---- internal DRAM ----
    h1t_in = nc.dram_tensor("h1t_in", [D, T], BF16)
    h1t_ag = nc.dram_tensor("h1t_ag", [N * D, T], BF16, addr_space="Shared")
    o_a2a_in = nc.dram_tensor("o_a2a_in", [N * 128, T], BF16)
    o_a2a_out = nc.dram_tensor("o_a2a_out", [N * 128, T], BF16)

    rg = [list(range(N))]

    with tile.TileContext(nc) as tc, ExitStack() as stack:
        consts = stack.enter_context(tc.tile_pool(name="consts", bufs=1))
        persist = stack.enter_context(tc.tile_pool(name="persist", bufs=1))
        wpool = stack.enter_context(tc.tile_pool(name="wpool", bufs=1))

        # x first — it heads the critical path into norm1 + AG; chunked so
        # stats start on the first 128-token group immediately
        xm_sb = persist.tile([128, 4, D], F32)   # x, later mid (x + attn)
        for tt4 in range(4):
            nc.sync.dma_start(
                out=xm_sb[:, tt4, :],
                in_=x_in[:].rearrange("(t p) d -> p t d", p=128)[:, tt4, :])

        ones_sb = consts.tile([1, 64], BF16)
        nc.vector.memset(ones_sb, 1.0)
        ident_sb = consts.tile([128, 128], BF16)
        nc.sync.dma_start(out=ident_sb, in_=ident_in[:])
        eps_sb = consts.tile([128, 1], F32)
        nc.vector.memset(eps_sb, EPS)

        # ---- norm helper: src [128,4,D] f32 -> hT tile [128,8,T] bf16 ----
        # transposes ordered dch-pair-major so D-chunks finish early (AG overlap)
        def rmsnorm_transpose(src_sb, npool, nps, hT_own, chunk_done=None):
            hts = []
            for tt4 in range(4):
                xsl = src_sb[:, tt4, :]
                stats = npool.tile([128, 2, 6], F32, tag="stats")
                nc.vector.bn_stats(out=stats[:, 0, :], in_=xsl[:, 0:512])
                nc.vector.bn_stats(out=stats[:, 1, :], in_=xsl[:, 512:1024])
                mv = npool.tile([128, 2], F32, tag="mv")
                nc.vector.bn_aggr(out=mv, in_=stats)
                msq = npool.tile([128, 1], F32, tag="msq")
                nc.vector.tensor_mul(out=msq, in0=mv[:, 0:1], in1=mv[:, 0:1])
                nc.vector.tensor_add(out=msq, in0=msq, in1=mv[:, 1:2])
                rstd = npool.tile([128, 1], F32, tag="rstd")
                nc.scalar.activation(out=rstd, in_=msq, func=AF.Sqrt, bias=eps_sb)
                nc.vector.reciprocal(out=rstd, in_=rstd)
                h_t = npool.tile([128, D], BF16, tag=f"h_t{tt4}")
                nc.vector.tensor_scalar_mul(out=h_t, in0=xsl, scalar1=rstd)
                hts.append(h_t)
            for c in range(2):
                for dch in range(4 * c, 4 * c + 4):
                    for tt4 in range(4):
                        ps_t = nps.tile([128, 128], BF16, tag="ps_t")
                        nc.tensor.transpose(out=ps_t, in_=hts[tt4][:, ts(dch, 128)],
                                            identity=ident_sb)
                        nc.vector.tensor_copy(out=hT_own[:, dch, ts(tt4, 128)],
                                              in_=ps_t)
                if chunk_done is not None:
                    chunk_done(c, hT_own)

        qT_sb = persist.tile([128, B * S], BF16)
        kT_sb = persist.tile([128, B * S], BF16)
        v_sb = persist.tile([128, 32, 130], BF16)
        oT_sb = persist.tile([128, B * S], BF16)

        # ================= phase 1: norm1 + AG =================
        with (
            tc.tile_pool(name="norm1", bufs=2) as npool1,
            tc.tile_pool(name="norm1_ps", bufs=2, space="PSUM") as nps1,
        ):
            h1T_own = npool1.tile([128, 8, T], BF16, tag="hT_own")
            rmsnorm_transpose(xm_sb, npool1, nps1, h1T_own)
            nc.sync.dma_start(
                out=h1t_in[:].rearrange("(c p) t -> p c t", p=128), in_=h1T_own)
        nc.gpsimd.collective_compute(
            "AllGather", mybir.AluOpType.bypass, replica_groups=rg,
            ins=[h1t_in[:]], outs=[h1t_ag[:]])

        # weight/const loads land here: they execute during the AllGather
        trigC_sb = consts.tile([128, S], F32)
        nc.sync.dma_start(out=trigC_sb, in_=trigC[:])
        trigS_sb = consts.tile([128, S], F32)
        nc.sync.dma_start(out=trigS_sb, in_=trigS[:])
        mask_sb = consts.tile([128, 128], BF16)
        nc.sync.dma_start(out=mask_sb, in_=mask_in[:])
        perm_sb = consts.tile([128, 128], BF16)
        nc.sync.dma_start(out=perm_sb, in_=perm_in[:])

        def load_w(name, dram, cols):
            t = wpool.tile([128, 8, cols], BF16, tag=name)
            nc.scalar.dma_start(out=t, in_=dram[:].rearrange("(c p) f -> p c f", p=128))
            return t

        wqT_sb = load_w("wqT", wqT, 128)
        wkT_sb = load_w("wkT", wkT, 128)
        wvT_sb = load_w("wvT", wvT, 128)
        woT_sb = load_w("woT", woT, D)

        with tc.tile_pool(name="big", bufs=1) as big:
            hT_sb = big.tile([128, 8, B * S], BF16, tag="big")
            for j in range(N):
                nc.sync.dma_start(
                    out=hT_sb[:, :, ts(j, T)],
                    in_=h1t_ag[ts(j, D), :].rearrange("(c p) t -> p c t", p=128))

            # ================= phase 2: QKV + RoPE =================
            with (
                tc.tile_pool(name="qkv_ps", bufs=2, space="PSUM") as qkv_ps,
                tc.tile_pool(name="rope", bufs=2) as rope,
            ):
                for tt in range(8):
                    pos = (tt % 4) * 512
                    for dst_sb, wT_t in ((qT_sb, wqT_sb), (kT_sb, wkT_sb)):
                        ps_x = qkv_ps.tile([128, 512], F32, tag="psx")
                        for dch in range(8):
                            nc.tensor.matmul(out=ps_x, lhsT=wT_t[:, dch, :],
                                             rhs=hT_sb[:, dch, ts(tt, 512)],
                                             start=dch == 0, stop=dch == 7)
                        # swapped-rows copy via permutation matmul (E<->O halves)
                        x_bf = rope.tile([128, 512], BF16, tag="x_bf")
                        nc.vector.tensor_copy(out=x_bf, in_=ps_x)
                        ps_xs = qkv_ps.tile([128, 512], F32, tag="psxs")
                        nc.tensor.matmul(out=ps_xs, lhsT=perm_sb, rhs=x_bf,
                                         start=True, stop=True)
                        t1 = rope.tile([128, 512], F32, tag="r1")
                        nc.vector.tensor_mul(out=t1, in0=ps_x,
                                             in1=trigC_sb[:, ds(pos, 512)])
                        t2 = rope.tile([128, 512], F32, tag="r2")
                        nc.vector.tensor_mul(out=t2, in0=ps_xs,
                                             in1=trigS_sb[:, ds(pos, 512)])
                        nc.vector.tensor_add(out=dst_sb[:, ts(tt, 512)],
                                             in0=t1, in1=t2)
                    for st in range(4):
                        tg = tt * 4 + st
                        ps_v = qkv_ps.tile([128, 128], F32, tag="psv")
                        for dch in range(8):
                            nc.tensor.matmul(
                                out=ps_v,
                                lhsT=hT_sb[:, dch, ds(tt * 512 + st * 128, 128)],
                                rhs=wvT_sb[:, dch, :],
                                start=dch == 0, stop=dch == 7)
                        nc.vector.tensor_copy(out=v_sb[:, tg, 0:64], in_=ps_v[:, 0:64])
                        nc.vector.tensor_copy(out=v_sb[:, tg, 65:129], in_=ps_v[:, 64:128])
                nc.vector.memset(v_sb[:, :, 64:65], 1.0)
                nc.vector.memset(v_sb[:, :, 129:130], 1.0)

        # ================= phase 3: attention =================
        # score matmuls packed: head0 at array rows 0-63, head1 at rows 64-127
        # (concurrent row-tiled MMs); exp over both heads in one ACT op.
        with (
            tc.tile_pool(name="pair_ps", bufs=2, space="PSUM") as pair_ps,
            tc.tile_pool(name="psum_o", bufs=2, space="PSUM") as psum_o,
            tc.tile_pool(name="attn_sb", bufs=3) as attn_sb,
        ):
            # normalization is emitted one block late so the PE never stalls
            # on the DVE reciprocal chain (stalls let HAM re-throttle).
            def emit_norm(blk):
                qbase_p, ps_o_p = blk
                bc = pair_ps.tile([128, 2, 512], F32, tag="pair", name="bc")
                for h in range(2):
                    # reciprocal_approx_fast (custom DVE op) misreads APs
                    # whose base partition is 64 — stage the denominator
                    # row into a base-0 SBUF tile first.
                    den_f = attn_sb.tile([1, 512], F32, tag=f"den_f{h}",
                                         name="den_f")
                    nc.vector.tensor_copy(out=den_f, in_=ps_o_p[h][64:65, :])
                    rec_f = attn_sb.tile([1, 512], F32, tag=f"rec_f{h}",
                                         name="rec_f")
                    nc.vector.reciprocal_approx_fast(out=rec_f, in_=den_f)
                    rec_b = attn_sb.tile([1, 512], BF16, tag=f"rec_b{h}",
                                         name="rec_b")
                    nc.vector.tensor_copy(out=rec_b, in_=rec_f)
                    nc.tensor.matmul(out=bc[64 * h:64 * h + 64, 0, :],
                                     lhsT=ones_sb, rhs=rec_b,
                                     start=True, stop=True)
                bc_sb = attn_sb.tile([128, 512], F32, tag="bc_sb")
                nc.vector.tensor_copy(out=bc_sb, in_=bc[:, 0, :])
                for h in range(2):
                    nc.vector.tensor_mul(
                        out=oT_sb[64 * h:64 * h + 64, ds(qbase_p, 512)],
                        in0=ps_o_p[h][0:64, :], in1=bc_sb[64 * h:64 * h + 64, :])

            prev_blk = None
            for b in range(B):
                for qt in range(4):
                    qbase = b * S + qt * 512
                    ps_o = [psum_o.tile([65, 512], F32, tag=f"ps_o{h}",
                                        name=f"ps_o{h}")
                            for h in range(2)]
                    nkt = 4 * qt + 4
                    for kt in range(nkt):
                        d_off = kt * 128 - qt * 512
                        c0 = max(d_off, 0)
                        pair = pair_ps.tile([128, 2, 512], F32, tag="pair")
                        for h in range(2):
                            nc.tensor.matmul(
                                out=pair[:, h, c0:512],
                                lhsT=kT_sb[64 * h:64 * h + 64,
                                           ds(b * S + kt * 128, 128)],
                                rhs=qT_sb[64 * h:64 * h + 64, ds(qbase + c0, 512 - c0)],
                                start=True, stop=True)
                        pT = attn_sb.tile([128, 2, 512], BF16, tag="pT")
                        nc.scalar.activation(out=pT[:, :, c0:512],
                                             in_=pair[:, :, c0:512], func=AF.Exp)
                        if d_off >= 0:
                            for h in range(2):
                                nc.vector.tensor_mul(
                                    out=pT[:, h, ds(d_off, 128)],
                                    in0=pT[:, h, ds(d_off, 128)], in1=mask_sb)
                        for h in range(2):
                            nc.tensor.matmul(
                                out=ps_o[h][:, c0:512],
                                lhsT=v_sb[:, b * 16 + kt, 65 * h:65 * h + 65],
                                rhs=pT[:, h, c0:512],
                                start=kt == 0, stop=kt == nkt - 1)
                        if kt == 1 and prev_blk is not None:
                            emit_norm(prev_blk)
                            prev_blk = None
                    prev_blk = (qbase, ps_o)
            emit_norm(prev_blk)

        # ================= phase 4: A2A of head outputs =================
        for j in range(N):
            nc.sync.dma_start(out=o_a2a_in[ts(j, 128), :], in_=oT_sb[:, ts(j, T)])
        nc.gpsimd.collective_compute(
            "AllToAll", mybir.AluOpType.bypass, replica_groups=rg,
            ins=[o_a2a_in[:]], outs=[o_a2a_out[:]])
        oag_sb = persist.tile([128, 8, T], BF16)
        nc.sync.dma_start(out=oag_sb,
                          in_=o_a2a_out[:].rearrange("(c p) t -> p c t", p=128))

        # ================= phase 5: o-proj + residual =================
        with tc.tile_pool(name="op_ps", bufs=2, space="PSUM") as op_ps:
            for tc4 in range(4):
                for n in range(2):
                    ps_op = op_ps.tile([128, 512], F32, tag="ps_op")
                    for fch in range(8):
                        nc.tensor.matmul(out=ps_op,
                                         lhsT=oag_sb[:, fch, ts(tc4, 128)],
                                         rhs=woT_sb[:, fch, ts(n, 512)],
                                         start=fch == 0, stop=fch == 7)
                    nc.vector.tensor_add(out=xm_sb[:, tc4, ts(n, 512)],
                                         in0=xm_sb[:, tc4, ts(n, 512)], in1=ps_op)

        # ================= phase 6: norm2 (local only, token-sharded FFN) ====
        with tc.tile_pool(name="ffn", bufs=1) as ffn_pool:
            h2T_sb = ffn_pool.tile([128, 8, T], BF16, tag="h2T")
            with (
                tc.tile_pool(name="norm2", bufs=2) as npool2,
                tc.tile_pool(name="norm2_ps", bufs=2, space="PSUM") as nps2,
            ):
                rmsnorm_transpose(xm_sb, npool2, nps2, h2T_sb)

            # ============ phase 7: FFN up + SwiGLU (full dff, own tokens) ====
            sT_sb = ffn_pool.tile([128, 32, T], BF16, tag="sT")
            with (
                tc.tile_pool(name="ffn_ps", bufs=2, space="PSUM") as ffn_ps,
                tc.tile_pool(name="wstream", bufs=3) as wstream,
                tc.tile_pool(name="ffn_sb", bufs=3) as ffn_sb,
            ):
                for dc in range(32):
                    w1c = wstream.tile([128, 8, 128], BF16, tag="w1c")
                    nc.sync.dma_start(
                        out=w1c,
                        in_=w1T[:, ts(dc, 128)].rearrange("(c p) f -> p c f", p=128))
                    w3c = wstream.tile([128, 8, 128], BF16, tag="w3c")
                    nc.sync.dma_start(
                        out=w3c,
                        in_=w3T[:, ts(dc, 128)].rearrange("(c p) f -> p c f", p=128))
                    ps_u = ffn_ps.tile([128, 512], F32, tag="ps_u")
                    for dch in range(8):
                        nc.tensor.matmul(out=ps_u, lhsT=w1c[:, dch, :],
                                         rhs=h2T_sb[:, dch, :],
                                         start=dch == 0, stop=dch == 7)
                    ps_g = ffn_ps.tile([128, 512], F32, tag="ps_g")
                    for dch in range(8):
                        nc.tensor.matmul(out=ps_g, lhsT=w3c[:, dch, :],
                                         rhs=h2T_sb[:, dch, :],
                                         start=dch == 0, stop=dch == 7)
                    silu_t = ffn_sb.tile([128, 512], F32, tag="silu")
                    nc.scalar.activation(out=silu_t, in_=ps_u, func=AF.Silu)
                    nc.vector.tensor_mul(out=sT_sb[:, dc, :], in0=silu_t, in1=ps_g)

            # ================= phase 8: down-proj + residual =================
            with (
                tc.tile_pool(name="dn_ps", bufs=1, space="PSUM") as dn_ps,
                tc.tile_pool(name="dn_sb", bufs=8) as dn_sb,
            ):
                ps_d = [dn_ps.tile([128, 512], F32, tag=f"ps_d{i}", name=f"ps_d{i}")
                        for i in range(8)]
                for dc in range(32):
                    for n in range(2):
                        w2c = dn_sb.tile([128, 512], BF16, tag=f"w2c{n}", name="w2c")
                        nc.scalar.dma_start(out=w2c, in_=w2T[ts(dc, 128), ts(n, 512)])
                        for tc4 in range(4):
                            nc.tensor.matmul(out=ps_d[n * 4 + tc4],
                                             lhsT=sT_sb[:, dc, ts(tc4, 128)],
                                             rhs=w2c,
                                             start=dc == 0, stop=dc == 31)
                for n in range(2):
                    for tc4 in range(4):
                        o_t = dn_sb.tile([128, 512], F32, tag="o_t")
                        nc.vector.tensor_add(out=o_t, in0=xm_sb[:, tc4, ts(n, 512)],
                                             in1=ps_d[n * 4 + tc4])
                        nc.sync.dma_start(
                            out=out_ext[:].rearrange("(t p) d -> p t d", p=128)[:, tc4, ts(n, 512)],
                            in_=o_t)

    nc.compile()
    return nc


def _host_prep(inputs):
    x = np.asarray(inputs["x"], np.float32).reshape(B * S, D)
    w_q = np.asarray(inputs["w_q"], np.float32)
    w_k = np.asarray(inputs["w_k"], np.float32)
    w_v = np.asarray(inputs["w_v"], np.float32)
    w_o = np.asarray(inputs["w_o"], np.float32)
    ln1 = np.asarray(inputs["ln1_w"], np.float32)
    ln2 = np.asarray(inputs["ln2_w"], np.float32)
    w1 = np.asarray(inputs["w1"], np.float32)
    w2 = np.asarray(inputs["w2"], np.float32)
    w3 = np.asarray(inputs["w3"], np.float32)

    wq_f = (w_q * ln1[None, :]) / np.sqrt(DK)
    wk_f = w_k * ln1[None, :]
    wv_f = w_v * ln1[None, :]
    w1_f = w1 * ln2[None, :]
    w3_f = w3 * ln2[None, :]

    # RoPE feature permutation: per core rows [h0E, h0O, h1E, h1O]
    jj = np.arange(32)
    swap_rows = np.concatenate([jj + 32, jj, jj + 96, jj + 64])
    # perm matmul matrix: out[m] = in[swap_rows[m]] -> P[k, m] = 1 iff k = swap(m)
    perm_mat = np.zeros((128, 128), dtype=BF16NP)
    perm_mat[swap_rows, np.arange(128)] = 1.0

    inv_freq = THETA ** (-(np.arange(0, DK, 2, dtype=np.float32) / DK))
    t_pos = np.arange(S, dtype=np.float32)
    ang = inv_freq[:, None] * t_pos[None, :]          # [32, S]
    c32, s32 = np.cos(ang), np.sin(ang)
    trigC = np.concatenate([c32, c32, c32, c32]).astype(np.float32)
    trigS = np.concatenate([-s32, s32, -s32, s32]).astype(np.float32)

    ident = np.eye(128, dtype=BF16NP)
    k_idx = np.arange(128)[:, None]
    q_idx = np.arange(128)[None, :]
    mask = (q_idx >= k_idx).astype(BF16NP)

    woT = np.ascontiguousarray(w_o.T).astype(BF16NP)
    w1T_full = np.ascontiguousarray(w1_f.T).astype(BF16NP)   # [D, DFF]
    w3T_full = np.ascontiguousarray(w3_f.T).astype(BF16NP)   # [D, DFF]
    w2T_full = np.ascontiguousarray(w2.T).astype(BF16NP)     # [DFF, D]

    in_maps = []
    for i in range(N):
        perm = []
        for h in range(HPC):
            base = (HPC * i + h) * DK
            perm.extend(base + 2 * jj)       # even
            perm.extend(base + 2 * jj + 1)   # odd
        perm = np.array(perm)
        wq_p = wq_f[perm]                    # [128, 1024]
        wk_p = wk_f[perm]
        wqT_i = np.ascontiguousarray(wq_p.T).astype(BF16NP)
        wkT_i = np.ascontiguousarray(wk_p.T).astype(BF16NP)
        in_maps.append({
            "x": np.ascontiguousarray(x[i * T:(i + 1) * T]),
            "wqT": wqT_i,
            "wkT": wkT_i,
            "rope_perm": perm_mat,
            "ident": ident,
            "wvT": np.ascontiguousarray(wv_f[i * 128:(i + 1) * 128].T).astype(BF16NP),
            "woT": woT,
            "w1T": w1T_full,
            "w3T": w3T_full,
            "w2T": w2T_full,
            "trigC": trigC,
            "trigS": trigS,
            "mask": mask,
        })
    return in_maps


def _get_nc():
    global _NC
    if _NC is None:
        _NC = _build()
    return _NC


def run(inputs, trace=False):
    nc = _get_nc()
    in_maps = _host_prep(inputs)
    res = run_bass_kernel_spmd(nc, in_maps, list(range(N)), trace=trace)
    out = np.concatenate([res.results[i]["out"] for i in range(N)], axis=0)
    return out.reshape(B, S, D).astype(np.float32), res


def kernel(**inputs):
    out, _ = run(inputs)
    return out
